# revision 3
# baseline (speedup 1.0000x reference)
"""Bass/Trainium2 kernel for nn_MAC_30554397344312 (gnn_message_passing).

Reference computation (B=256 rollout groups, n=64 agents, D=256):
    comm = h @ W_act.T + b_act                      # (B*n, D)
    agg[b,j] = sum_i mask[i,j] * comm[b,i] / (n-1)  # mask = ones - eye
    x   = agg @ W_sum.T + b_sum
    out = relu(x @ W_head.T + b_head)

Everything before the relu is linear, so fold on host:
    Wc = W_head @ W_sum @ W_act          (256x256)
    bc = b_head + b_sum @ W_head.T + b_act @ (W_head @ W_sum).T
    out[b,j] = relu( (A @ H_b)[j] @ Wc.T + bc ),  A = ones-eye (1/63 in Wc)

Per core (2048 rows = 16 token tiles), a chunked pipeline:
    load (2-tile chunks; sync ring takes the first 6, scalar ring the
        last 2 so the load stream finishes early and stores queue
        right behind on the scalar ring)
    cast f32->fp16 on GpSimd (otherwise idle engine)
    stage 1 (PE fp16): Y.T tiles [d, tok] via matmul(lhsT=H[128t,128d],
        rhs=blockdiag(A,A)) - aggregation and transpose fused
    evict Y.T PSUM -> SBUF fp16 on DVE ([128,512] ops)
    stage 3 (PE fp16): out[tok, dout] = Y.T.T @ Wc.T over 2 k-chunks
    relu+descale evict, alternating DVE / ACT
    store (2-tile chunks on the scalar ring)

Sharding: data-parallel over the B axis, 8 cores x 32 groups.
"""

from contextlib import ExitStack

import numpy as np

import concourse.bacc as bacc
import concourse.bass as bass
import concourse.tile as tile
from concourse import mybir
from concourse.bass_utils import run_bass_kernel_spmd

N_AGENTS = 64
B = 256
D = 256
N_CORES = 8
ROWS = B * N_AGENTS            # 16384
ROWS_PER_CORE = ROWS // N_CORES  # 2048
P = 128
N_TILES = ROWS_PER_CORE // P   # 16 token tiles per core
LC = 2                         # tiles per load/store chunk (256 KiB)
N_LCHUNKS = N_TILES // LC      # 8
BT = 4                         # tiles per agg/main batch
N_BATCH = N_TILES // BT        # 4
W_SCALE = 16.0  # fp16 weight prescale (power of 2; inverted exactly in relu)

_cache = {}


def _build(has_bias: bool):
    f32 = mybir.dt.float32
    f16 = mybir.dt.float16
    inv_scale = 1.0 / W_SCALE
    nc = bacc.Bacc("TRN2", target_bir_lowering=False, debug=False,
                   num_devices=N_CORES)

    h = nc.dram_tensor("h", [ROWS_PER_CORE, D], f32, kind="ExternalInput")
    wcT = nc.dram_tensor("wcT", [D, D], f16, kind="ExternalInput")
    ablk = nc.dram_tensor("ablk", [P, P], f16, kind="ExternalInput")
    if has_bias:
        bc = nc.dram_tensor("bc", [1, D], f32, kind="ExternalInput")
    out = nc.dram_tensor("out", [ROWS_PER_CORE, D], f32, kind="ExternalOutput")

    h_ap = h[:, :].rearrange("(n p) d -> p n d", p=P)      # [128, 16, 256]
    out_ap = out[:, :].rearrange("(n p) d -> p n d", p=P)  # [128, 16, 256]

    with tile.TileContext(nc) as tc:
        with ExitStack() as ctx:
            const = ctx.enter_context(tc.tile_pool(name="const", bufs=1))
            ytps = ctx.enter_context(
                tc.tile_pool(name="ytps", bufs=3, space="PSUM"))
            outps = ctx.enter_context(
                tc.tile_pool(name="outps", bufs=4, space="PSUM"))

            # sync ring: mask first (gates stage 1), then h chunks 0..5
            a_t = const.tile([P, P], f16, tag="a", name="a_t")
            nc.sync.dma_start(out=a_t[:], in_=ablk[:, :])
            traw = [const.tile([P, LC, D], f32, tag=f"hr{c}", name=f"hr_{c}")
                    for c in range(N_LCHUNKS)]
            for c in range(N_LCHUNKS - 2):
                nc.sync.dma_start(out=traw[c][:],
                                  in_=h_ap[:, c * LC:(c + 1) * LC, :])
            # scalar ring: weights (needed ~mid-kernel), tail h chunks,
            # then the stores queue behind
            w_t = [const.tile([P, D], f16, tag=f"w{k}", name=f"w_{k}")
                   for k in range(2)]
            for k in range(2):
                nc.scalar.dma_start(out=w_t[k][:],
                                    in_=wcT[k * P:(k + 1) * P, :])
            for c in range(N_LCHUNKS - 2, N_LCHUNKS):
                nc.scalar.dma_start(out=traw[c][:],
                                    in_=h_ap[:, c * LC:(c + 1) * LC, :])
            if has_bias:
                bc_t = const.tile([P, D], f32, tag="bc", name="bc_t")
                bc_bcast = bass.AP(tensor=bc, offset=0, ap=[[0, P], [1, D]])
                nc.gpsimd.dma_start(out=bc_t[:], in_=bc_bcast)

            # fp16 h tiles (GpSimd casts), Y.T, and store staging
            hc = [const.tile([P, LC, D], f16, tag=f"hc{c}", name=f"hc_{c}")
                  for c in range(N_LCHUNKS)]
            yt = [const.tile([P, ROWS_PER_CORE], f16, tag=f"yt{k}",
                             name=f"yt_{k}") for k in range(2)]
            och = [const.tile([P, LC, D], f32, tag=f"oc{c}", name=f"oc_{c}")
                   for c in range(N_LCHUNKS)]

            def agg_batch(b):
                for c in (2 * b, 2 * b + 1):
                    nc.gpsimd.tensor_copy(out=hc[c][:], in_=traw[c][:])
                ps = [ytps.tile([P, BT * P], f32, tag="ytps", name="yt_ps")
                      for _ in range(2)]
                for s in range(BT):
                    c, r = divmod(b * BT + s, LC)
                    for k in range(2):
                        lhsT = hc[c][:, r, k * P:(k + 1) * P]
                        nc.tensor.matmul(
                            ps[k][:, s * P:(s + 1) * P], lhsT, a_t[:],
                            start=True, stop=True)
                for k in range(2):
                    nc.vector.tensor_copy(
                        yt[k][:, b * BT * P:(b + 1) * BT * P], ps[k][:])

            def main_batch(b):
                for half in range(BT // LC):
                    c = b * (BT // LC) + half           # 2-tile store chunk
                    po = outps.tile([P, LC * D], f32, tag="outps", name="po")
                    for s in range(LC):
                        m = c * LC + s
                        for k in range(2):
                            nc.tensor.matmul(
                                po[:, s * D:(s + 1) * D],
                                yt[k][:, m * P:(m + 1) * P], w_t[k][:],
                                start=(k == 0), stop=(k == 1))
                    dst = och[c][:, :, :]
                    if has_bias:
                        for s in range(LC):
                            d1 = och[c][:, s, :]
                            nc.vector.tensor_scalar(
                                out=d1, in0=po[:, s * D:(s + 1) * D],
                                scalar1=inv_scale, scalar2=None,
                                op0=mybir.AluOpType.mult)
                            nc.vector.tensor_tensor(
                                out=d1, in0=d1, in1=bc_t[:],
                                op=mybir.AluOpType.add)
                            nc.scalar.activation(
                                out=d1, in_=d1,
                                func=mybir.ActivationFunctionType.Relu)
                    elif c % 2 == 0:
                        nc.vector.tensor_scalar(
                            out=dst, in0=po[:], scalar1=inv_scale,
                            scalar2=0.0, op0=mybir.AluOpType.mult,
                            op1=mybir.AluOpType.max)
                    else:
                        nc.scalar.activation(
                            out=dst, in_=po[:],
                            func=mybir.ActivationFunctionType.Relu,
                            scale=inv_scale)
                    nc.scalar.dma_start(
                        out=out_ap[:, c * LC:(c + 1) * LC, :], in_=och[c][:])

            agg_batch(0)
            agg_batch(1)
            main_batch(0)
            agg_batch(2)
            main_batch(1)
            agg_batch(3)
            main_batch(2)
            main_batch(3)
    nc.finalize()
    return nc


def _fold(W_act, b_act, W_sum, b_sum, W_head, b_head):
    Wa = W_act.astype(np.float64)
    Ws = W_sum.astype(np.float64)
    Wh = W_head.astype(np.float64)
    Wc = Wh @ Ws @ Wa
    bc = (b_head.astype(np.float64)
          + b_sum.astype(np.float64) @ Wh.T
          + b_act.astype(np.float64) @ (Wh @ Ws).T)
    A = np.ones((N_AGENTS, N_AGENTS)) - np.eye(N_AGENTS)
    # mask stays exact 0/1 in fp16; 1/63 and the fp16-subnormal
    # prescale fold into the weights, inverted via the relu scale.
    WcT = (Wc.T / (N_AGENTS - 1) * W_SCALE).astype(np.float16)
    Ablk = np.zeros((P, P), np.float16)
    Ablk[:N_AGENTS, :N_AGENTS] = A
    Ablk[N_AGENTS:, N_AGENTS:] = A
    return np.ascontiguousarray(WcT), bc.astype(np.float32), Ablk


def kernel(hidden_state, W_act, b_act, W_sum, b_sum, W_head, b_head,
           _trace=False, _tmpdir=None):
    h = np.ascontiguousarray(np.asarray(hidden_state, dtype=np.float32))
    WcT, bc, Ablk = _fold(np.asarray(W_act), np.asarray(b_act),
                          np.asarray(W_sum), np.asarray(b_sum),
                          np.asarray(W_head), np.asarray(b_head))
    has_bias = bool(np.any(bc))
    if has_bias not in _cache:
        _cache[has_bias] = _build(has_bias)
    nc = _cache[has_bias]

    in_maps = []
    for c in range(N_CORES):
        m = {"h": h[c * ROWS_PER_CORE:(c + 1) * ROWS_PER_CORE],
             "wcT": WcT, "ablk": Ablk}
        if has_bias:
            m["bc"] = bc.reshape(1, D)
        in_maps.append(m)

    res = run_bass_kernel_spmd(
        nc, in_maps, core_ids=list(range(N_CORES)),
        trace=_trace, tmpdir=_tmpdir)
    out = np.concatenate([res.results[c]["out"] for c in range(N_CORES)],
                         axis=0)
    if _trace:
        return out, res
    return out


# revision 4
# speedup vs baseline: 1.1308x; 1.1308x over previous
"""Bass/Trainium2 kernel for nn_MAC_30554397344312 (gnn_message_passing).

Reference computation (B=256 rollout groups, n=64 agents, D=256):
    comm = h @ W_act.T + b_act                      # (B*n, D)
    agg[b,j] = sum_i mask[i,j] * comm[b,i] / (n-1)  # mask = ones - eye
    x   = agg @ W_sum.T + b_sum
    out = relu(x @ W_head.T + b_head)

Everything before the relu is linear, so fold on host:
    Wc = W_head @ W_sum @ W_act          (256x256)
    bc = b_head + b_sum @ W_head.T + b_act @ (W_head @ W_sum).T
    out[b,j] = relu( (A @ H_b)[j] @ Wc.T + bc ),  A = ones-eye (1/63 in Wc)

Per core (2048 rows = 16 token tiles), a chunked pipeline:
    load (2-tile chunks; sync ring takes the first 6, scalar ring the
        last 2 so the load stream finishes early and stores queue
        right behind on the scalar ring)
    cast f32->fp16 on GpSimd (otherwise idle engine)
    stage 1 (PE fp16): Y.T tiles [d, tok] via matmul(lhsT=H[128t,128d],
        rhs=blockdiag(A,A)) - aggregation and transpose fused
    evict Y.T PSUM -> SBUF fp16 on DVE ([128,512] ops)
    stage 3 (PE fp16): out[tok, dout] = Y.T.T @ Wc.T over 2 k-chunks
    relu+descale evict, alternating DVE / ACT
    store (2-tile chunks on the scalar ring)

Sharding: data-parallel over the B axis, 8 cores x 32 groups.
"""

from contextlib import ExitStack

import numpy as np

import concourse.bacc as bacc
import concourse.bass as bass
import concourse.tile as tile
from concourse import mybir
from concourse.bass_utils import run_bass_kernel_spmd

N_AGENTS = 64
B = 256
D = 256
N_CORES = 8
ROWS = B * N_AGENTS            # 16384
ROWS_PER_CORE = ROWS // N_CORES  # 2048
P = 128
N_TILES = ROWS_PER_CORE // P   # 16 token tiles per core
LC = 2                         # tiles per load/store chunk (256 KiB)
N_LCHUNKS = N_TILES // LC      # 8
BT = 4                         # tiles per agg/main batch
N_BATCH = N_TILES // BT        # 4
W_SCALE = 16.0  # fp16 weight prescale (power of 2; inverted exactly in relu)

_cache = {}


def _build(has_bias: bool):
    f32 = mybir.dt.float32
    f16 = mybir.dt.float16
    inv_scale = 1.0 / W_SCALE
    nc = bacc.Bacc("TRN2", target_bir_lowering=False, debug=False,
                   num_devices=N_CORES)

    h = nc.dram_tensor("h", [ROWS_PER_CORE, D], f32, kind="ExternalInput")
    wcT = nc.dram_tensor("wcT", [D, D], f16, kind="ExternalInput")
    ablk = nc.dram_tensor("ablk", [P, P], f16, kind="ExternalInput")
    if has_bias:
        bc = nc.dram_tensor("bc", [1, D], f32, kind="ExternalInput")
    out = nc.dram_tensor("out", [ROWS_PER_CORE, D], f32, kind="ExternalOutput")

    h_ap = h[:, :].rearrange("(n p) d -> p n d", p=P)      # [128, 16, 256]
    out_ap = out[:, :].rearrange("(n p) d -> p n d", p=P)  # [128, 16, 256]

    with tile.TileContext(nc) as tc:
        with ExitStack() as ctx:
            const = ctx.enter_context(tc.tile_pool(name="const", bufs=1))
            ytps = ctx.enter_context(
                tc.tile_pool(name="ytps", bufs=3, space="PSUM"))
            outps = ctx.enter_context(
                tc.tile_pool(name="outps", bufs=4, space="PSUM"))

            # sync ring: mask first (gates stage 1), then h chunks 0..5
            a_t = const.tile([P, P], f16, tag="a", name="a_t")
            nc.sync.dma_start(out=a_t[:], in_=ablk[:, :])
            traw = [const.tile([P, LC, D], f32, tag=f"hr{c}", name=f"hr_{c}")
                    for c in range(N_LCHUNKS)]
            for c in range(N_LCHUNKS - 2):
                nc.sync.dma_start(out=traw[c][:],
                                  in_=h_ap[:, c * LC:(c + 1) * LC, :])
            # scalar ring: weights (needed ~mid-kernel), tail h chunks,
            # then the stores queue behind
            w_t = [const.tile([P, D], f16, tag=f"w{k}", name=f"w_{k}")
                   for k in range(2)]
            for k in range(2):
                nc.scalar.dma_start(out=w_t[k][:],
                                    in_=wcT[k * P:(k + 1) * P, :])
            for c in range(N_LCHUNKS - 2, N_LCHUNKS):
                nc.scalar.dma_start(out=traw[c][:],
                                    in_=h_ap[:, c * LC:(c + 1) * LC, :])
            if has_bias:
                bc_t = const.tile([P, D], f32, tag="bc", name="bc_t")
                bc_bcast = bass.AP(tensor=bc, offset=0, ap=[[0, P], [1, D]])
                nc.gpsimd.dma_start(out=bc_t[:], in_=bc_bcast)

            # fp16 h tiles (GpSimd casts), Y.T, and store staging
            hc = [const.tile([P, LC, D], f16, tag=f"hc{c}", name=f"hc_{c}")
                  for c in range(N_LCHUNKS)]
            yt = [const.tile([P, ROWS_PER_CORE], f16, tag=f"yt{k}",
                             name=f"yt_{k}") for k in range(2)]
            och = [const.tile([P, LC, D], f32, tag=f"oc{c}", name=f"oc_{c}")
                   for c in range(N_LCHUNKS)]

            def agg_batch(b):
                for c in (2 * b, 2 * b + 1):
                    if c % 2 == 0:
                        nc.vector.tensor_copy(out=hc[c][:], in_=traw[c][:])
                    else:
                        nc.scalar.activation(
                            out=hc[c][:], in_=traw[c][:],
                            func=mybir.ActivationFunctionType.Copy)
                ps = [ytps.tile([P, BT * P], f32, tag="ytps", name="yt_ps")
                      for _ in range(2)]
                for s in range(BT):
                    c, r = divmod(b * BT + s, LC)
                    for k in range(2):
                        lhsT = hc[c][:, r, k * P:(k + 1) * P]
                        nc.tensor.matmul(
                            ps[k][:, s * P:(s + 1) * P], lhsT, a_t[:],
                            start=True, stop=True)
                for k in range(2):
                    dst = yt[k][:, b * BT * P:(b + 1) * BT * P]
                    if b % 2 == 1 and k == 1:
                        nc.scalar.activation(
                            out=dst, in_=ps[k][:],
                            func=mybir.ActivationFunctionType.Copy)
                    else:
                        nc.vector.tensor_copy(dst, ps[k][:])

            def main_batch(b):
                for half in range(BT // LC):
                    c = b * (BT // LC) + half           # 2-tile store chunk
                    po = outps.tile([P, LC * D], f32, tag="outps", name="po")
                    for s in range(LC):
                        m = c * LC + s
                        for k in range(2):
                            nc.tensor.matmul(
                                po[:, s * D:(s + 1) * D],
                                yt[k][:, m * P:(m + 1) * P], w_t[k][:],
                                start=(k == 0), stop=(k == 1))
                    dst = och[c][:, :, :]
                    if has_bias:
                        for s in range(LC):
                            d1 = och[c][:, s, :]
                            nc.vector.tensor_scalar(
                                out=d1, in0=po[:, s * D:(s + 1) * D],
                                scalar1=inv_scale, scalar2=None,
                                op0=mybir.AluOpType.mult)
                            nc.vector.tensor_tensor(
                                out=d1, in0=d1, in1=bc_t[:],
                                op=mybir.AluOpType.add)
                            nc.scalar.activation(
                                out=d1, in_=d1,
                                func=mybir.ActivationFunctionType.Relu)
                    elif c % 2 == 0:
                        nc.vector.tensor_scalar(
                            out=dst, in0=po[:], scalar1=inv_scale,
                            scalar2=0.0, op0=mybir.AluOpType.mult,
                            op1=mybir.AluOpType.max)
                    else:
                        nc.scalar.activation(
                            out=dst, in_=po[:],
                            func=mybir.ActivationFunctionType.Relu,
                            scale=inv_scale)
                    nc.scalar.dma_start(
                        out=out_ap[:, c * LC:(c + 1) * LC, :], in_=och[c][:])

            agg_batch(0)
            agg_batch(1)
            main_batch(0)
            agg_batch(2)
            main_batch(1)
            agg_batch(3)
            main_batch(2)
            main_batch(3)
    nc.finalize()
    return nc


def _fold(W_act, b_act, W_sum, b_sum, W_head, b_head):
    Wa = W_act.astype(np.float64)
    Ws = W_sum.astype(np.float64)
    Wh = W_head.astype(np.float64)
    Wc = Wh @ Ws @ Wa
    bc = (b_head.astype(np.float64)
          + b_sum.astype(np.float64) @ Wh.T
          + b_act.astype(np.float64) @ (Wh @ Ws).T)
    A = np.ones((N_AGENTS, N_AGENTS)) - np.eye(N_AGENTS)
    # mask stays exact 0/1 in fp16; 1/63 and the fp16-subnormal
    # prescale fold into the weights, inverted via the relu scale.
    WcT = (Wc.T / (N_AGENTS - 1) * W_SCALE).astype(np.float16)
    Ablk = np.zeros((P, P), np.float16)
    Ablk[:N_AGENTS, :N_AGENTS] = A
    Ablk[N_AGENTS:, N_AGENTS:] = A
    return np.ascontiguousarray(WcT), bc.astype(np.float32), Ablk


def kernel(hidden_state, W_act, b_act, W_sum, b_sum, W_head, b_head,
           _trace=False, _tmpdir=None):
    h = np.ascontiguousarray(np.asarray(hidden_state, dtype=np.float32))
    WcT, bc, Ablk = _fold(np.asarray(W_act), np.asarray(b_act),
                          np.asarray(W_sum), np.asarray(b_sum),
                          np.asarray(W_head), np.asarray(b_head))
    has_bias = bool(np.any(bc))
    if has_bias not in _cache:
        _cache[has_bias] = _build(has_bias)
    nc = _cache[has_bias]

    in_maps = []
    for c in range(N_CORES):
        m = {"h": h[c * ROWS_PER_CORE:(c + 1) * ROWS_PER_CORE],
             "wcT": WcT, "ablk": Ablk}
        if has_bias:
            m["bc"] = bc.reshape(1, D)
        in_maps.append(m)

    res = run_bass_kernel_spmd(
        nc, in_maps, core_ids=list(range(N_CORES)),
        trace=_trace, tmpdir=_tmpdir)
    out = np.concatenate([res.results[c]["out"] for c in range(N_CORES)],
                         axis=0)
    if _trace:
        return out, res
    return out


# revision 5
# speedup vs baseline: 1.3337x; 1.1794x over previous
"""Bass/Trainium2 kernel for nn_MAC_30554397344312 (gnn_message_passing).

Reference computation (B=256 rollout groups, n=64 agents, D=256):
    comm = h @ W_act.T + b_act                      # (B*n, D)
    agg[b,j] = sum_i mask[i,j] * comm[b,i] / (n-1)  # mask = ones - eye
    x   = agg @ W_sum.T + b_sum
    out = relu(x @ W_head.T + b_head)

Everything before the relu is linear, so fold on host:
    Wc = W_head @ W_sum @ W_act          (256x256)
    bc = b_head + b_sum @ W_head.T + b_act @ (W_head @ W_sum).T
    out[b,j] = relu( (A @ H_b)[j] @ Wc.T + bc ),  A = ones-eye (1/63 in Wc)

Per core (2048 rows = 16 token tiles), a chunked pipeline:
    load (2-tile chunks; sync ring takes the first 6, scalar ring the
        last 2 so the load stream finishes early and stores queue
        right behind on the scalar ring)
    cast f32->fp16 on DVE (issued ahead of the evicts)
    stage 1 (PE fp16): Y.T tiles [d, tok] via matmul(lhsT=H[128t,128d],
        rhs=blockdiag(A,A)) - aggregation and transpose fused
    evict Y.T PSUM -> SBUF fp16 on DVE ([128,512] ops)
    stage 3 (PE fp16): out[tok, dout] = Y.T.T @ Wc.T over 2 k-chunks
    relu+descale evict on ACT
    store (2-tile chunks on the scalar ring)

Sharding: data-parallel over the B axis, 8 cores x 32 groups.
"""

from contextlib import ExitStack

import numpy as np

import concourse.bacc as bacc
import concourse.bass as bass
import concourse.tile as tile
from concourse import mybir
from concourse.bass_utils import run_bass_kernel_spmd

N_AGENTS = 64
B = 256
D = 256
N_CORES = 8
ROWS = B * N_AGENTS            # 16384
ROWS_PER_CORE = ROWS // N_CORES  # 2048
P = 128
N_TILES = ROWS_PER_CORE // P   # 16 token tiles per core
LC = 2                         # tiles per load/store chunk (256 KiB)
N_LCHUNKS = N_TILES // LC      # 8
BT = 4                         # tiles per agg/main batch
N_BATCH = N_TILES // BT        # 4
W_SCALE = 16.0  # fp16 weight prescale (power of 2; inverted exactly in relu)

_cache = {}


def _build(has_bias: bool):
    f32 = mybir.dt.float32
    f16 = mybir.dt.float16
    inv_scale = 1.0 / W_SCALE
    nc = bacc.Bacc("TRN2", target_bir_lowering=False, debug=False,
                   num_devices=N_CORES)

    h = nc.dram_tensor("h", [ROWS_PER_CORE, D], f32, kind="ExternalInput")
    wcT = nc.dram_tensor("wcT", [D, D], f16, kind="ExternalInput")
    ablk = nc.dram_tensor("ablk", [P, P], f16, kind="ExternalInput")
    if has_bias:
        bc = nc.dram_tensor("bc", [1, D], f32, kind="ExternalInput")
    out = nc.dram_tensor("out", [ROWS_PER_CORE, D], f32, kind="ExternalOutput")

    h_ap = h[:, :].rearrange("(n p) d -> p n d", p=P)      # [128, 16, 256]
    out_ap = out[:, :].rearrange("(n p) d -> p n d", p=P)  # [128, 16, 256]

    with tile.TileContext(nc) as tc:
        with ExitStack() as ctx:
            const = ctx.enter_context(tc.tile_pool(name="const", bufs=1))
            ytps = ctx.enter_context(
                tc.tile_pool(name="ytps", bufs=3, space="PSUM"))
            outps = ctx.enter_context(
                tc.tile_pool(name="outps", bufs=4, space="PSUM"))

            # sync ring: mask first (gates stage 1), then h chunks 0..5
            a_t = const.tile([P, P], f16, tag="a", name="a_t")
            nc.sync.dma_start(out=a_t[:], in_=ablk[:, :])
            traw = [const.tile([P, LC, D], f32, tag=f"hr{c}", name=f"hr_{c}")
                    for c in range(N_LCHUNKS)]
            for c in range(N_LCHUNKS - 2):
                nc.sync.dma_start(out=traw[c][:],
                                  in_=h_ap[:, c * LC:(c + 1) * LC, :])
            # scalar ring: tail h chunks, weights (needed only by stage 3),
            # then the stores queue behind
            for c in range(N_LCHUNKS - 2, N_LCHUNKS):
                nc.scalar.dma_start(out=traw[c][:],
                                    in_=h_ap[:, c * LC:(c + 1) * LC, :])
            w_t = [const.tile([P, D], f16, tag=f"w{k}", name=f"w_{k}")
                   for k in range(2)]
            for k in range(2):
                nc.scalar.dma_start(out=w_t[k][:],
                                    in_=wcT[k * P:(k + 1) * P, :])
            if has_bias:
                bc_t = const.tile([P, D], f32, tag="bc", name="bc_t")
                bc_bcast = bass.AP(tensor=bc, offset=0, ap=[[0, P], [1, D]])
                nc.gpsimd.dma_start(out=bc_t[:], in_=bc_bcast)

            # fp16 h tiles (GpSimd casts), Y.T, and store staging
            hc = [const.tile([P, LC, D], f16, tag=f"hc{c}", name=f"hc_{c}")
                  for c in range(N_LCHUNKS)]
            yt = [const.tile([P, ROWS_PER_CORE], f16, tag=f"yt{k}",
                             name=f"yt_{k}") for k in range(2)]
            och = [const.tile([P, LC, D], f32, tag=f"oc{c}", name=f"oc_{c}")
                   for c in range(N_LCHUNKS)]

            def cast_chunk(c):
                nc.vector.tensor_copy(out=hc[c][:], in_=traw[c][:])

            def agg_batch(b):
                ps = [ytps.tile([P, BT * P], f32, tag="ytps", name="yt_ps")
                      for _ in range(2)]
                for s in range(BT):
                    c, r = divmod(b * BT + s, LC)
                    for k in range(2):
                        lhsT = hc[c][:, r, k * P:(k + 1) * P]
                        nc.tensor.matmul(
                            ps[k][:, s * P:(s + 1) * P], lhsT, a_t[:],
                            start=True, stop=True)
                for k in range(2):
                    nc.vector.tensor_copy(
                        yt[k][:, b * BT * P:(b + 1) * BT * P], ps[k][:])

            def main_batch(b):
                for half in range(BT // LC):
                    c = b * (BT // LC) + half           # 2-tile store chunk
                    po = outps.tile([P, LC * D], f32, tag="outps", name="po")
                    for s in range(LC):
                        m = c * LC + s
                        for k in range(2):
                            nc.tensor.matmul(
                                po[:, s * D:(s + 1) * D],
                                yt[k][:, m * P:(m + 1) * P], w_t[k][:],
                                start=(k == 0), stop=(k == 1))
                    dst = och[c][:, :, :]
                    if has_bias:
                        for s in range(LC):
                            d1 = och[c][:, s, :]
                            nc.vector.tensor_scalar(
                                out=d1, in0=po[:, s * D:(s + 1) * D],
                                scalar1=inv_scale, scalar2=None,
                                op0=mybir.AluOpType.mult)
                            nc.vector.tensor_tensor(
                                out=d1, in0=d1, in1=bc_t[:],
                                op=mybir.AluOpType.add)
                            nc.scalar.activation(
                                out=d1, in_=d1,
                                func=mybir.ActivationFunctionType.Relu)
                    else:
                        nc.scalar.activation(
                            out=dst, in_=po[:],
                            func=mybir.ActivationFunctionType.Relu,
                            scale=inv_scale)
                    nc.scalar.dma_start(
                        out=out_ap[:, c * LC:(c + 1) * LC, :], in_=och[c][:])

            # software-pipelined issue order: casts run ahead so the DVE
            # queue never blocks a ready cast behind a PE-dependent evict
            cast_chunk(0)
            cast_chunk(1)
            cast_chunk(2)
            cast_chunk(3)
            agg_batch(0)
            cast_chunk(4)
            cast_chunk(5)
            agg_batch(1)
            main_batch(0)
            cast_chunk(6)
            cast_chunk(7)
            agg_batch(2)
            main_batch(1)
            agg_batch(3)
            main_batch(2)
            main_batch(3)
    nc.finalize()
    return nc


def _fold(W_act, b_act, W_sum, b_sum, W_head, b_head):
    Wa = W_act.astype(np.float64)
    Ws = W_sum.astype(np.float64)
    Wh = W_head.astype(np.float64)
    Wc = Wh @ Ws @ Wa
    bc = (b_head.astype(np.float64)
          + b_sum.astype(np.float64) @ Wh.T
          + b_act.astype(np.float64) @ (Wh @ Ws).T)
    A = np.ones((N_AGENTS, N_AGENTS)) - np.eye(N_AGENTS)
    # mask stays exact 0/1 in fp16; 1/63 and the fp16-subnormal
    # prescale fold into the weights, inverted via the relu scale.
    WcT = (Wc.T / (N_AGENTS - 1) * W_SCALE).astype(np.float16)
    Ablk = np.zeros((P, P), np.float16)
    Ablk[:N_AGENTS, :N_AGENTS] = A
    Ablk[N_AGENTS:, N_AGENTS:] = A
    return np.ascontiguousarray(WcT), bc.astype(np.float32), Ablk


def kernel(hidden_state, W_act, b_act, W_sum, b_sum, W_head, b_head,
           _trace=False, _tmpdir=None):
    h = np.ascontiguousarray(np.asarray(hidden_state, dtype=np.float32))
    WcT, bc, Ablk = _fold(np.asarray(W_act), np.asarray(b_act),
                          np.asarray(W_sum), np.asarray(b_sum),
                          np.asarray(W_head), np.asarray(b_head))
    has_bias = bool(np.any(bc))
    if has_bias not in _cache:
        _cache[has_bias] = _build(has_bias)
    nc = _cache[has_bias]

    in_maps = []
    for c in range(N_CORES):
        m = {"h": h[c * ROWS_PER_CORE:(c + 1) * ROWS_PER_CORE],
             "wcT": WcT, "ablk": Ablk}
        if has_bias:
            m["bc"] = bc.reshape(1, D)
        in_maps.append(m)

    res = run_bass_kernel_spmd(
        nc, in_maps, core_ids=list(range(N_CORES)),
        trace=_trace, tmpdir=_tmpdir)
    out = np.concatenate([res.results[c]["out"] for c in range(N_CORES)],
                         axis=0)
    if _trace:
        return out, res
    return out
